# revision 50
# baseline (speedup 1.0000x reference)
"""Trainium2 Bass kernel for nn_ButterflyLayer2D (butterfly 2D CNN).

Strategy: pure data parallel over 8 NeuronCores (16 batch each).

Layouts (per core, bf16 activations):
  - conv-level inputs live in SBUF as [128 = (w%2)*64 + c, (node, b, h, w2)]
    so each 2x2-stride-2 per-node conv is computed with the q-scheme:
    output-w-parity q lands in psum partition half q via col-masked M=64
    matmuls at tile_position (0, 64q) (the two q matmuls run concurrently
    on the PE), x (h-parity) accumulates in PSUM.  K = (y, ci) = 128.
  - psum tiles then map 1:1 onto the next level's layout: partition
    (q, co) -> (w%2, c), cols (b, ho, w2o) -> (b, h, w2).  Every eviction
    is a single full-width 128-partition op with contiguous reads and
    writes (relu+bias fused via ScalarE activation / VectorE tensor_scalar,
    alternating engines).
  - deep levels (4, 5) batch many nodes into one [128, 1024] psum tile and
    evict with a 2-pass batched op (tensor_tensor add with a broadcast
    per-node bias AP, then relu) to amortize per-op overheads.
  - level 6 packs node pairs: psum [128=(s, c), (pair, b)]; dense reads the
    resulting F [128=(s, c), (pair, b)] with row-tiled K=64 matmuls
    (tile_position (64s, 0)) writing [128=(r,ou,ov), (pair, b)] per s.
  - the input 4x4-patch conv uses K=16 row-groups spread over 4 partition
    groups (one per b%4) x 2 col groups for 8-way PE tile concurrency.
  - ~20 warm-up matmuls on a memset tile run at t=0 (concurrent with input
    DMA) so the PE HAM clock-gate reaches 8/8 before the real work starts.
Weights are streamed from HBM in chunks through recycled tile tags; DMAs
are issued in consumption order with the input staged first.
"""

import numpy as np
from contextlib import ExitStack

import concourse.bass as bass
import concourse.tile as tile
from concourse import bacc, mybir
from concourse.bass_utils import run_bass_kernel_spmd

F32 = mybir.dt.float32
BF16 = mybir.dt.bfloat16
AF = mybir.ActivationFunctionType
ALU = mybir.AluOpType

B, IN, NLVL, KLVL, C = 128, 256, 6, 3, 64
TCOL = 1024               # psum tile columns
PBUFS = 4                 # psum tile bufs
NK, OU, OV = 8, 8, 8
NCORES = 8
BC = B // NCORES          # 16 per-core batch
PH = 1                    # phases per core
BG = BC // PH             # batch per phase
HALF = BG // 4            # input-conv b-subgroups per partition group
LVL_NODES = [4, 16, 64, 64, 64, 64]          # nodes per level
LVL_HIN = [64, 32, 16, 8, 4, 2]              # spatial H into each level
NWARM = 18                # HAM warm-up matmuls (cover until the input DMA lands)


# ----------------------------------------------------------------------------
# host-side pre-arrangement
# ----------------------------------------------------------------------------

def _prep_weights(inputs):
    """Weights/biases blobs shared by all cores."""
    import ml_dtypes
    out = {}
    # input filter: lhsT [16=(p,q), 64], replicated at partition bases 0/32/64/96
    # K=32 y-slot pairing: rows (g, yslot, p*4+q), cols (yslot', co) —
    # block-diagonal so two output rows' y-parities come out of one matmul
    fin = inputs["in_filter"][:, :, 0, :].reshape(16, C).astype(np.float32)
    finr = np.zeros((128, 2 * C), np.float32)
    for g in range(4):
        for ys in range(2):
            finr[g * 32 + ys * 16 : g * 32 + (ys + 1) * 16,
                 ys * C : (ys + 1) * C] = fin
    out["fin"] = finr.astype(ml_dtypes.bfloat16)
    out["bin"] = np.concatenate([inputs["in_bias"], inputs["in_bias"]]).reshape(
        128, 1
    ).astype(np.float32)

    for lvl in range(1, NLVL + 1):
        f = inputs[f"f{lvl}"].astype(np.float32)  # [n,n,2,2,C,C] (x,y,ci,co)
        n = f.shape[0]
        n2 = n * n
        # per node lhsT [(y*64+ci), (x*64+co)], node-major blob
        w = f.transpose(0, 1, 3, 4, 2, 5).reshape(n2, 2 * C, 2 * C)
        out[f"w{lvl}"] = np.ascontiguousarray(w.transpose(1, 0, 2)).reshape(
            128, n2 * 128
        ).astype(ml_dtypes.bfloat16)
        b = inputs[f"b{lvl}"].astype(np.float32).reshape(n2, C)
        if lvl < NLVL:
            # [128, nodes]: rows (q,c) with bias duplicated across q
            bb = np.concatenate([b, b], axis=1)  # [nodes, 128]
            out[f"b{lvl}"] = np.ascontiguousarray(bb.T)
        else:
            # level-6 bias broadcast blob [128=(s,c), (pair, b)]
            bb = b.reshape(n2 // 2, 2, C)            # [pair, s, c]
            bb = bb.transpose(1, 2, 0)               # [s, c, pair]
            bb = np.repeat(bb.reshape(128, n2 // 2, 1), BG, axis=2)
            out["b6bc"] = np.ascontiguousarray(
                bb.reshape(128, n2 // 2 * BG)
            ).astype(ml_dtypes.bfloat16)
    # dense: blob [128=(s*64+c), (pair, r*64+ou*8+ov)]
    wd = inputs["Wd"].astype(np.float32).reshape(NK * NK, 2, C, OU * OV)
    wd = wd.reshape(NK * NK // 2, 2, 2, C, OU * OV)   # [pair, s, r, c, k]
    wd = wd.transpose(1, 3, 0, 2, 4)                  # [s, c, pair, r, k]
    out["wd"] = np.ascontiguousarray(wd.reshape(128, NK * NK // 2 * 128)).astype(
        ml_dtypes.bfloat16
    )
    return out


def _prep_input(in_data_core):
    """Per-core input blob: [128 = g*32 + (j//4)%2*16 + (i%4)*4 + (j%4),
    (ph, h2, s, x=i//4, y8=j//8)] with b = h2*8 + g*2 + s: K=32 matmuls
    produce both y-parities at once, and each g-chunk DMA covers complete
    consecutive b-pairs."""
    import ml_dtypes
    ind = in_data_core[:, :, :, 0]  # [16, 256, 256]
    a = ind.reshape(PH, 2, 4, 2, 64, 4, 32, 2, 4)  # ph,h2,g,s,x,p,y8,ys,q
    a = a.transpose(2, 7, 5, 8, 0, 1, 3, 4, 6)     # g,ys,p,q,ph,h2,s,x,y8
    return np.ascontiguousarray(a).reshape(128, PH * HALF * 64 * 32).astype(ml_dtypes.bfloat16)


def _decode_output(t2_core):
    """t2 [128=(r,ou,ov), (ph, node, bl)] -> [16, 64, 64, 2]."""
    t = t2_core.reshape(2, OU, OV, PH, NK, NK, BG)  # r,ou,ov,ph,u,v,bl
    t = t.transpose(3, 6, 4, 1, 5, 2, 0)            # ph,bl,u,ou,v,ov,r
    return np.ascontiguousarray(t).reshape(BC, NK * OU, NK * OV, 2)


# ----------------------------------------------------------------------------
# device kernel
# ----------------------------------------------------------------------------

def _build_kernel():
    nc = bacc.Bacc(None, target_bir_lowering=False)
    p = {}
    p["a0"] = nc.declare_dram_parameter("a0", [128, PH * HALF * 64 * 32], BF16, isOutput=False)
    p["fin"] = nc.declare_dram_parameter("fin", [128, 2 * C], BF16, isOutput=False)
    p["bin"] = nc.declare_dram_parameter("bin", [128, 1], F32, isOutput=False)
    for lvl in range(1, NLVL + 1):
        n2 = LVL_NODES[lvl - 1]
        p[f"w{lvl}"] = nc.declare_dram_parameter(f"w{lvl}", [128, n2 * 128], BF16, isOutput=False)
        if lvl < NLVL:
            p[f"b{lvl}"] = nc.declare_dram_parameter(f"b{lvl}", [128, n2], F32, isOutput=False)
    p["b6bc"] = nc.declare_dram_parameter("b6bc", [128, 32 * BG], BF16, isOutput=False)
    p["wd"] = nc.declare_dram_parameter("wd", [128, 32 * 128], BF16, isOutput=False)
    t2 = nc.declare_dram_parameter("t2", [128, PH * NK * NK * BG], F32, isOutput=True)

    evict_ctr = [0]

    def evict(out_ap, psum_ap, bias_ap):
        """relu(psum + bias) -> sbuf, alternating engines to split the load."""
        evict_ctr[0] += 1
        # ACT is 1.25x faster per element than DVE on TRN2: give it 5 of 9
        if evict_ctr[0] % 9 in (0, 2, 4, 6, 8):
            nc.scalar.activation(out_ap, psum_ap, AF.Relu, bias=bias_ap)
        else:
            nc.vector.tensor_scalar(out_ap, psum_ap, bias_ap, 0.0,
                                    op0=ALU.add, op1=ALU.max)

    with tile.TileContext(nc) as tc, ExitStack() as ctx:
        const = ctx.enter_context(tc.tile_pool(name="const", bufs=1))
        wpool = ctx.enter_context(tc.tile_pool(name="wts", bufs=5))
        apool = ctx.enter_context(tc.tile_pool(name="acts", bufs=1))
        inpool = ctx.enter_context(tc.tile_pool(name="inp", bufs=1))
        fpool = ctx.enter_context(tc.tile_pool(name="feat", bufs=1))
        tpool = ctx.enter_context(tc.tile_pool(name="tmp", bufs=2))
        ppool = ctx.enter_context(tc.tile_pool(name="ps", bufs=PBUFS, space="PSUM"))

        # ------------- HAM warm-up (runs while input DMA streams) -------------
        wm = const.tile([128, 512], BF16, name="warm")
        nc.vector.memset(wm[:], 0.0)
        for i in range(NWARM):
            ptw = ppool.tile([128, 256], F32, tag="ps", padded_shape=[128, TCOL],
                             name=f"warm{i}")
            nc.tensor.matmul(ptw[:], wm[:, 0:128], wm[:, 0:256],
                             start=True, stop=True)

        # ------------- input + constant DMAs (consumption order) -------------
        fin_t = const.tile([128, 2 * C], BF16)
        nc.sync.dma_start(fin_t[:], p["fin"][:])
        bin_t = const.tile([128, 1], F32)
        nc.sync.dma_start(bin_t[:], p["bin"][:])
        # a0 lands as 8 quarter-transfers, h2-major: after the first four, all
        # four g row-groups have their h2=0 batch halves — so the input conv
        # can run 4 matmul streams on independent PE row-tiles immediately.
        a0s = inpool.tile([128, PH * HALF * 64 * 32], BF16, tag="a0s", name="a0s")
        hcols = PH * HALF * 64 * 16        # half the cols = h2=0 part
        for g in range(4):
            nc.sync.dma_start(a0s[g * 32 : (g + 1) * 32, 0:hcols],
                              p["a0"][g * 32 : (g + 1) * 32, 0:hcols])
        # w1 rides between the a0 chunks so L1 can start right after the input
        w1t = const.tile([128, 4 * 128], BF16, name="w1t")
        nc.sync.dma_start(w1t[:], p["w1"][:])
        bias_t = {}
        bias_t[1] = const.tile([128, 4], F32, tag="bias1", name="bias1")
        nc.sync.dma_start(bias_t[1][:], p["b1"][:])
        for g in range(4):
            nc.sync.dma_start(
                a0s[g * 32 : (g + 1) * 32, hcols:],
                p["a0"][g * 32 : (g + 1) * 32, hcols:],
            )
        for lvl in range(2, NLVL):
            bias_t[lvl] = const.tile([128, LVL_NODES[lvl - 1]], F32,
                                     tag=f"bias{lvl}", name=f"bias{lvl}")
            nc.sync.dma_start(bias_t[lvl][:], p[f"b{lvl}"][:])
        b6bc_t = const.tile([128, 32 * BG], BF16, name="b6bc")
        nc.sync.dma_start(b6bc_t[:], p["b6bc"][:])
        wdt = const.tile([128, 32 * 128], BF16, name="wd")  # DMA'd during L2

        ph = 0
        a0v = a0s[:].rearrange("p (h x y) -> p h x y", h=HALF, x=64)

        # ---------------- input conv (K=32, chunk-major) ---------------------
        # X slab: [128=(y%2,c), (b, h=64, w2=32)]; b = h2*8 + g*2 + s
        X = apool.tile([128, BG * 64 * 32], BF16, tag="s0", name="x0")
        Xv = X[:].rearrange("p (b h w) -> p b h w", b=BG, h=64)
        a1 = apool.tile([128, 4 * BG * 32 * 16], BF16, tag="s1", name="a1")
        a1v = a1[:].rearrange("p (n b h w) -> p n b h w", n=4, b=BG, h=32)

        def in_quad(bls):
            # four bls on four distinct g row-groups: matmuls interleave
            # round-robin so all four PE row-tiles stream concurrently
            for xh in range(2):
                pts = {}
                for bl in bls:
                    pts[bl] = ppool.tile([128, TCOL], F32, tag="ps",
                                         padded_shape=[128, TCOL],
                                         name=f"pin{bl}_{xh}")
                for sub in range(2):
                    for bl in bls:
                        g, hi = (bl // 2) % 4, (bl // 8) * 2 + bl % 2
                        xq = xh * 2 + sub
                        rhs = a0v[g * 32 : (g + 1) * 32, hi,
                                  xq * 16 : (xq + 1) * 16, :]
                        nc.tensor.matmul(
                            pts[bl][:, sub * 512 : (sub + 1) * 512],
                            fin_t[g * 32 : (g + 1) * 32, :],
                            rhs,
                            start=True, stop=True,
                            tile_position=(g * 32, 0),
                        )
                for bl in bls:
                    # split the evict across both engines: halves the latency
                    # so the psum ring never gates the matmul stream
                    nc.scalar.activation(
                        Xv[:, bl, xh * 32 : xh * 32 + 16, :],
                        pts[bl][:, 0:512],
                        AF.Relu, bias=bin_t[:, 0:1])
                    nc.vector.tensor_scalar(
                        Xv[:, bl, xh * 32 + 16 : (xh + 1) * 32, :],
                        pts[bl][:, 512:1024],
                        bin_t[:, 0:1], 0.0, op0=ALU.add, op1=ALU.max)

        def l1_tiles(bs):
            for node in range(4):
                pt = ppool.tile([128, TCOL], F32, tag="ps",
                                padded_shape=[128, TCOL],
                                name=f"p1_{node}_{bs}")
                for x in (0, 1):
                    for q in (0, 1):
                        for sb in range(2):
                            rhs = Xv[:, bs + sb, x::2, q::2]
                            nc.tensor.matmul(
                                pt[q * 64 : (q + 1) * 64,
                                   sb * 512 : (sb + 1) * 512],
                                w1t[:, node * 128 + x * 64 :
                                    node * 128 + (x + 1) * 64],
                                rhs,
                                start=(x == 0), stop=(x == 1),
                                skip_group_check=True,
                                tile_position=(0, q * 64),
                            )
                evict(a1v[:, node, bs : bs + 2, :, :], pt[:],
                      bias_t[1][:, node : node + 1])

        # h2=0 quads first (unlocked by the first four quarter-transfers)
        for bls in ((0, 2, 4, 6), (1, 3, 5, 7), (8, 10, 12, 14), (9, 11, 13, 15)):
            in_quad(bls)
        for bs in range(0, BG, 2):
            l1_tiles(bs)

        # ---------------- levels 2..3 (q-scheme, per-node psum) --------------
        cur, cur_nodes = a1, 4
        tags = [None, "s0", "s1"]
        for lvl in (2, 3):
            n2 = LVL_NODES[lvl - 1]
            grid = int(np.sqrt(n2))
            pgrid = int(np.sqrt(cur_nodes))
            Hin = LVL_HIN[lvl - 1]
            W2in = Hin // 2
            Ho, Ko = Hin // 2, W2in // 2      # psum cols per b = Ho*Ko
            ncolb = Ho * Ko
            bper = min(BG, TCOL // ncolb)
            nsub = (bper * ncolb) // 512       # 512-col chunks per tile
            bsub = bper // nsub
            nxt = apool.tile([128, n2 * BG * ncolb], BF16,
                             tag=tags[lvl - 1], name=f"a{lvl}")
            curv = cur[:].rearrange("p (n b h w) -> p n b h w",
                                    n=cur_nodes, b=BG, h=Hin)
            nxtv = nxt[:].rearrange("p (n b h w) -> p n b h w",
                                    n=n2, b=BG, h=Ho)
            if lvl == 3:
                # dense weights: issued here so the 1 MB transfer drains
                # during L2 compute, well before anything needs the queue
                nc.sync.dma_start(wdt[:], p["wd"][:])
            # stream this level's weights in one or two chunks
            wchunk = min(n2, 16)
            for g0 in range(0, n2, wchunk):
                wlt = wpool.tile([128, 16 * 128], BF16, tag="wch",
                                 name=f"w{lvl}_{g0}")
                nc.sync.dma_start(
                    wlt[:, : wchunk * 128],
                    p[f"w{lvl}"][:, g0 * 128 : (g0 + wchunk) * 128],
                )
                # npt: nodes per psum tile (pair nodes when a node is <= 512)
                npt = 2 if bper * ncolb <= 512 else 1
                ntc = bper * ncolb                 # cols per node in the tile
                for n0 in range(g0, g0 + wchunk, npt):
                    for bs in range(0, BG, bper):
                        pt = ppool.tile([128, npt * ntc], F32, tag="ps",
                                        padded_shape=[128, TCOL],
                                        name=f"p{lvl}_{n0}_{bs}")
                        for node in range(n0, n0 + npt):
                            u, v = node // grid, node % grid
                            pn = (u // 2) * pgrid + (v // 2)
                            ln = node - g0
                            lo = (node - n0) * ntc
                            for x in (0, 1):
                                for q in (0, 1):
                                    for sb in range(nsub):
                                        b1 = bs + sb * bsub
                                        rhs = curv[:, pn, b1 : b1 + bsub,
                                                   x::2, q::2]
                                        nc.tensor.matmul(
                                            pt[q * 64 : (q + 1) * 64,
                                               lo + sb * 512 :
                                               lo + (sb + 1) * 512],
                                            wlt[:, ln * 128 + x * 64 :
                                                ln * 128 + (x + 1) * 64],
                                            rhs,
                                            start=(x == 0), stop=(x == 1),
                                            skip_group_check=True,
                                            tile_position=(0, q * 64),
                                        )
                        for node in range(n0, n0 + npt):
                            lo = (node - n0) * ntc
                            evict(
                                nxtv[:, node, bs : bs + bper, :, :],
                                pt[:, lo : lo + ntc],
                                bias_t[lvl][:, node : node + 1],
                            )
            cur, cur_nodes = nxt, n2

        # ---------------- levels 4..5 (q-scheme, node-batched psum) ----------
        for lvl in (4, 5):
            n2 = 64
            Hin = LVL_HIN[lvl - 1]
            W2in = Hin // 2
            Ho, Ko = Hin // 2, W2in // 2
            ncoln = BG * Ho * max(Ko, 1)       # cols per node (Ko>=1)
            gper = min(TCOL // ncoln, 16)      # nodes per psum tile
            nxt = apool.tile([128, n2 * ncoln], BF16,
                             tag=("s0" if lvl == 4 else "s1"), name=f"a{lvl}")
            curv = cur[:].rearrange("p (n b h w) -> p n b h w",
                                    n=64, b=BG, h=Hin)
            nxtv = nxt[:].rearrange("p (n c) -> p n c", n=n2)
            for g0 in range(0, n2, 16):
                wlt = wpool.tile([128, 16 * 128], BF16, tag="wch",
                                 name=f"w{lvl}_{g0}")
                nc.sync.dma_start(
                    wlt[:], p[f"w{lvl}"][:, g0 * 128 : (g0 + 16) * 128]
                )
                for t0 in range(g0, g0 + 16, gper):
                    pt = ppool.tile([128, gper * ncoln], F32, tag="ps",
                                    padded_shape=[128, TCOL],
                                    name=f"p{lvl}_{t0}")
                    for node in range(t0, t0 + gper):
                        ln, lt = node - g0, node - t0
                        for x in (0, 1):
                            for q in (0, 1):
                                rhs = curv[:, node, :, x::2, q::2]
                                nc.tensor.matmul(
                                    pt[q * 64 : (q + 1) * 64,
                                       lt * ncoln : (lt + 1) * ncoln],
                                    wlt[:, ln * 128 + x * 64 :
                                        ln * 128 + (x + 1) * 64],
                                    rhs,
                                    start=(x == 0), stop=(x == 1),
                                    skip_group_check=True,
                                    tile_position=(0, q * 64),
                                )
                    # batched 2-pass evict: add broadcast bias, then relu
                    tmp = tpool.tile([128, TCOL], BF16, tag="etmp",
                                     name=f"t{lvl}_{t0}")
                    bias_ap = bias_t[lvl][:, t0 : t0 + gper].unsqueeze(2) \
                        .broadcast_to([128, gper, ncoln])
                    ptv = pt[:].rearrange("p (n c) -> p n c", n=gper)
                    tv = tmp[:, : gper * ncoln].rearrange(
                        "p (n c) -> p n c", n=gper)
                    nc.vector.tensor_tensor(tv, ptv, bias_ap, op=ALU.add)
                    nc.scalar.activation(
                        nxtv[:, t0 : t0 + gper, :],
                        tv, AF.Relu,
                    )
            cur = nxt

        # ------- level 6 + dense + output, pipelined in two pair-halves ------
        # L6: node pairs -> F [128=(s,c), (pair, b)]; dense row-tiled K=64.
        F = fpool.tile([128, 32 * BG], BF16, tag="feats", name="f6")
        Fv = F[:].rearrange("p (n b) -> p n b", n=32)
        curv = cur[:].rearrange("p (n b h) -> p n b h", n=64, b=BG)
        t2s = fpool.tile([128, NK * NK * BG], F32, tag="t2s", name="t2s")
        t2sv = t2s[:].rearrange("m (n b) -> m n b", n=NK * NK)
        # hoist all w6 chunk DMAs so the second half never waits on weights
        w6ts = {}
        for g0 in range(0, 64, 16):
            w6ts[g0] = wpool.tile([128, 16 * 128], BF16, tag="wch",
                                  name=f"w6_{g0}")
            nc.sync.dma_start(
                w6ts[g0][:], p["w6"][:, g0 * 128 : (g0 + 16) * 128]
            )
        # both L6 halves' matmuls first (half 1 hides half 0's F eviction),
        # then the two dense halves (half 1 hides half 0's output copies)
        def l6_half(hf):
            pt6 = ppool.tile([128, 16 * BG], F32, tag="ps",
                             padded_shape=[128, TCOL], name=f"p6_{hf}")
            for g0 in (hf * 32, hf * 32 + 16):
                w6t = w6ts[g0]
                for node in range(g0, g0 + 16):
                    pr, s = node // 2, node % 2
                    ln = node - g0
                    lp = pr - hf * 16
                    for x in (0, 1):
                        rhs = curv[:, node, :, x]
                        nc.tensor.matmul(
                            pt6[s * 64 : (s + 1) * 64,
                                lp * BG : (lp + 1) * BG],
                            w6t[:, ln * 128 + x * 64 : ln * 128 + (x + 1) * 64],
                            rhs,
                            start=(x == 0), stop=(x == 1),
                            skip_group_check=True,
                            tile_position=(0, s * 64),
                        )
            tmp6 = tpool.tile([128, 16 * BG], BF16, tag="etmp", name=f"t6_{hf}")
            nc.vector.tensor_tensor(tmp6[:], pt6[:],
                                    b6bc_t[:, hf * 256 : (hf + 1) * 256],
                                    op=ALU.add)
            nc.scalar.activation(F[:, hf * 256 : (hf + 1) * 256], tmp6[:],
                                 AF.Relu)

        def dense_half(hf):
            ptd = {}
            for s in (0, 1):
                ptd[s] = ppool.tile([128, 16 * BG], F32, tag="ps",
                                    padded_shape=[128, TCOL], name=f"pd{hf}_{s}")
            for pr in range(hf * 16, hf * 16 + 16):
                lp = pr - hf * 16
                for s in (0, 1):
                    nc.tensor.matmul(
                        ptd[s][:, lp * BG : (lp + 1) * BG],
                        wdt[s * 64 : (s + 1) * 64, pr * 128 : (pr + 1) * 128],
                        Fv[s * 64 : (s + 1) * 64, pr, :],
                        start=True, stop=True,
                        tile_position=(s * 64, 0),
                    )
            for s in (0, 1):
                dst = t2sv[:, hf * 32 + s : (hf + 1) * 32 : 2, :]
                src = ptd[s][:].rearrange("m (n b) -> m n b", n=16)
                if s == 0:
                    nc.vector.tensor_copy(dst, src)
                else:
                    nc.scalar.copy(dst, src)
            nc.sync.dma_start(
                t2[:, hf * 512 : (hf + 1) * 512],
                t2s[:, hf * 512 : (hf + 1) * 512],
            )

        l6_half(0)
        l6_half(1)
        dense_half(0)
        dense_half(1)
    nc.compile()
    return nc


# ----------------------------------------------------------------------------
# entry point
# ----------------------------------------------------------------------------

def kernel(**inputs):
    inputs = {k: np.asarray(v) for k, v in inputs.items()}
    wblobs = _prep_weights(inputs)
    nc = _build_kernel()
    in_maps = []
    for c in range(NCORES):
        m = dict(wblobs)
        m["a0"] = _prep_input(inputs["in_data"][c * BC : (c + 1) * BC])
        in_maps.append(m)
    res = run_bass_kernel_spmd(nc, in_maps, list(range(NCORES)))
    outs = [_decode_output(res.results[c]["t2"]) for c in range(NCORES)]
    return np.concatenate(outs, axis=0).astype(np.float32)


if __name__ == "__main__":
    import reference as ref

    inputs = {k: np.asarray(v) for k, v in ref.setup_inputs().items()}
    expected = np.asarray(ref.reference(**inputs))
    actual = kernel(**inputs)
    err = np.abs(actual - expected).max()
    rel = err / np.abs(expected).max()
    print("absmax:", err, "rel:", rel)


# revision 51
# speedup vs baseline: 1.2118x; 1.2118x over previous
"""Trainium2 Bass kernel for nn_ButterflyLayer2D (butterfly 2D CNN).

Strategy: pure data parallel over 8 NeuronCores (16 batch each).

Layouts (per core, bf16 activations):
  - conv-level inputs live in SBUF as [128 = (w%2)*64 + c, (node, b, h, w2)]
    so each 2x2-stride-2 per-node conv is computed with the q-scheme:
    output-w-parity q lands in psum partition half q via col-masked M=64
    matmuls at tile_position (0, 64q) (the two q matmuls run concurrently
    on the PE), x (h-parity) accumulates in PSUM.  K = (y, ci) = 128.
  - psum tiles then map 1:1 onto the next level's layout: partition
    (q, co) -> (w%2, c), cols (b, ho, w2o) -> (b, h, w2).  Every eviction
    is a single full-width 128-partition op with contiguous reads and
    writes (relu+bias fused via ScalarE activation / VectorE tensor_scalar,
    alternating engines).
  - deep levels (4, 5) batch many nodes into one [128, 1024] psum tile and
    evict with a 2-pass batched op (tensor_tensor add with a broadcast
    per-node bias AP, then relu) to amortize per-op overheads.
  - level 6 packs node pairs: psum [128=(s, c), (pair, b)]; dense reads the
    resulting F [128=(s, c), (pair, b)] with row-tiled K=64 matmuls
    (tile_position (64s, 0)) writing [128=(r,ou,ov), (pair, b)] per s.
  - the input 4x4-patch conv uses K=16 row-groups spread over 4 partition
    groups (one per b%4) x 2 col groups for 8-way PE tile concurrency.
  - ~20 warm-up matmuls on a memset tile run at t=0 (concurrent with input
    DMA) so the PE HAM clock-gate reaches 8/8 before the real work starts.
Weights are streamed from HBM in chunks through recycled tile tags; DMAs
are issued in consumption order with the input staged first.
"""

import numpy as np
from contextlib import ExitStack

import concourse.bass as bass
import concourse.tile as tile
from concourse import bacc, mybir
from concourse.bass_utils import run_bass_kernel_spmd

F32 = mybir.dt.float32
BF16 = mybir.dt.bfloat16
AF = mybir.ActivationFunctionType
ALU = mybir.AluOpType

B, IN, NLVL, KLVL, C = 128, 256, 6, 3, 64
TCOL = 1024               # psum tile columns
PBUFS = 4                 # psum tile bufs
NK, OU, OV = 8, 8, 8
NCORES = 8
BC = B // NCORES          # 16 per-core batch
PH = 1                    # phases per core
BG = BC // PH             # batch per phase
HALF = BG // 4            # input-conv b-subgroups per partition group
LVL_NODES = [4, 16, 64, 64, 64, 64]          # nodes per level
LVL_HIN = [64, 32, 16, 8, 4, 2]              # spatial H into each level
NWARM = 18                # HAM warm-up matmuls (cover until the input DMA lands)


# ----------------------------------------------------------------------------
# host-side pre-arrangement
# ----------------------------------------------------------------------------

def _prep_weights(inputs):
    """Weights/biases blobs shared by all cores."""
    import ml_dtypes
    out = {}
    # input filter: lhsT [16=(p,q), 64], replicated at partition bases 0/32/64/96
    # K=32 y-slot pairing: rows (g, yslot, p*4+q), cols (yslot', co) —
    # block-diagonal so two output rows' y-parities come out of one matmul
    fin = inputs["in_filter"][:, :, 0, :].reshape(16, C).astype(np.float32)
    finr = np.zeros((128, 2 * C), np.float32)
    for g in range(4):
        for ys in range(2):
            finr[g * 32 + ys * 16 : g * 32 + (ys + 1) * 16,
                 ys * C : (ys + 1) * C] = fin
    out["fin"] = finr.astype(ml_dtypes.bfloat16)
    out["bin"] = np.concatenate([inputs["in_bias"], inputs["in_bias"]]).reshape(
        128, 1
    ).astype(np.float32)

    for lvl in range(1, NLVL + 1):
        f = inputs[f"f{lvl}"].astype(np.float32)  # [n,n,2,2,C,C] (x,y,ci,co)
        n = f.shape[0]
        n2 = n * n
        # per node lhsT [(y*64+ci), (x*64+co)], node-major blob
        w = f.transpose(0, 1, 3, 4, 2, 5).reshape(n2, 2 * C, 2 * C)
        out[f"w{lvl}"] = np.ascontiguousarray(w.transpose(1, 0, 2)).reshape(
            128, n2 * 128
        ).astype(ml_dtypes.bfloat16)
        b = inputs[f"b{lvl}"].astype(np.float32).reshape(n2, C)
        if lvl < NLVL:
            # [128, nodes]: rows (q,c) with bias duplicated across q
            bb = np.concatenate([b, b], axis=1)  # [nodes, 128]
            out[f"b{lvl}"] = np.ascontiguousarray(bb.T)
        else:
            # level-6 bias broadcast blob [128=(s,c), (pair, b)]
            bb = b.reshape(n2 // 2, 2, C)            # [pair, s, c]
            bb = bb.transpose(1, 2, 0)               # [s, c, pair]
            bb = np.repeat(bb.reshape(128, n2 // 2, 1), BG, axis=2)
            out["b6bc"] = np.ascontiguousarray(
                bb.reshape(128, n2 // 2 * BG)
            ).astype(ml_dtypes.bfloat16)
    # dense: blob [128=(s*64+c), (pair, r*64+ou*8+ov)]
    wd = inputs["Wd"].astype(np.float32).reshape(NK * NK, 2, C, OU * OV)
    wd = wd.reshape(NK * NK // 2, 2, 2, C, OU * OV)   # [pair, s, r, c, k]
    wd = wd.transpose(1, 3, 0, 2, 4)                  # [s, c, pair, r, k]
    out["wd"] = np.ascontiguousarray(wd.reshape(128, NK * NK // 2 * 128)).astype(
        ml_dtypes.bfloat16
    )
    return out


def _prep_input(in_data_core):
    """Per-core input blob: [128 = g*32 + (j//4)%2*16 + (i%4)*4 + (j%4),
    (ph, h2, s, x=i//4, y8=j//8)] with b = h2*8 + g*2 + s: K=32 matmuls
    produce both y-parities at once, and each g-chunk DMA covers complete
    consecutive b-pairs."""
    import ml_dtypes
    ind = in_data_core[:, :, :, 0]  # [16, 256, 256]
    a = ind.reshape(PH, 2, 4, 2, 64, 4, 32, 2, 4)  # ph,h2,g,s,x,p,y8,ys,q
    a = a.transpose(2, 7, 5, 8, 0, 1, 3, 4, 6)     # g,ys,p,q,ph,h2,s,x,y8
    return np.ascontiguousarray(a).reshape(128, PH * HALF * 64 * 32).astype(ml_dtypes.bfloat16)


def _decode_output(t2_core):
    """t2 [128=(r,ou,ov), (ph, node, bl)] -> [16, 64, 64, 2]."""
    t = t2_core.reshape(2, OU, OV, PH, NK, NK, BG)  # r,ou,ov,ph,u,v,bl
    t = t.transpose(3, 6, 4, 1, 5, 2, 0)            # ph,bl,u,ou,v,ov,r
    return np.ascontiguousarray(t).reshape(BC, NK * OU, NK * OV, 2)


# ----------------------------------------------------------------------------
# device kernel
# ----------------------------------------------------------------------------

def _build_kernel():
    nc = bacc.Bacc(None, target_bir_lowering=False)
    p = {}
    p["a0"] = nc.declare_dram_parameter("a0", [128, PH * HALF * 64 * 32], BF16, isOutput=False)
    p["fin"] = nc.declare_dram_parameter("fin", [128, 2 * C], BF16, isOutput=False)
    p["bin"] = nc.declare_dram_parameter("bin", [128, 1], F32, isOutput=False)
    for lvl in range(1, NLVL + 1):
        n2 = LVL_NODES[lvl - 1]
        p[f"w{lvl}"] = nc.declare_dram_parameter(f"w{lvl}", [128, n2 * 128], BF16, isOutput=False)
        if lvl < NLVL:
            p[f"b{lvl}"] = nc.declare_dram_parameter(f"b{lvl}", [128, n2], F32, isOutput=False)
    p["b6bc"] = nc.declare_dram_parameter("b6bc", [128, 32 * BG], BF16, isOutput=False)
    p["wd"] = nc.declare_dram_parameter("wd", [128, 32 * 128], BF16, isOutput=False)
    t2 = nc.declare_dram_parameter("t2", [128, PH * NK * NK * BG], F32, isOutput=True)

    evict_ctr = [0]

    def evict(out_ap, psum_ap, bias_ap):
        """relu(psum + bias) -> sbuf, alternating engines to split the load."""
        evict_ctr[0] += 1
        # ACT is 1.25x faster per element than DVE on TRN2: give it 5 of 9
        if evict_ctr[0] % 9 in (0, 2, 4, 6, 8):
            nc.scalar.activation(out_ap, psum_ap, AF.Relu, bias=bias_ap)
        else:
            nc.vector.tensor_scalar(out_ap, psum_ap, bias_ap, 0.0,
                                    op0=ALU.add, op1=ALU.max)

    with tile.TileContext(nc) as tc, ExitStack() as ctx:
        const = ctx.enter_context(tc.tile_pool(name="const", bufs=1))
        wpool = ctx.enter_context(tc.tile_pool(name="wts", bufs=5))
        apool = ctx.enter_context(tc.tile_pool(name="acts", bufs=1))
        inpool = ctx.enter_context(tc.tile_pool(name="inp", bufs=1))
        fpool = ctx.enter_context(tc.tile_pool(name="feat", bufs=1))
        tpool = ctx.enter_context(tc.tile_pool(name="tmp", bufs=2))
        ppool = ctx.enter_context(tc.tile_pool(name="ps", bufs=PBUFS, space="PSUM"))

        # ------------- HAM warm-up (runs while input DMA streams) -------------
        wm = const.tile([128, 512], BF16, name="warm")
        nc.vector.memset(wm[:], 0.0)
        for i in range(NWARM):
            ptw = ppool.tile([128, 256], F32, tag="ps", padded_shape=[128, TCOL],
                             name=f"warm{i}")
            nc.tensor.matmul(ptw[:], wm[:, 0:128], wm[:, 0:256],
                             start=True, stop=True)

        # ------------- input + constant DMAs (consumption order) -------------
        fin_t = const.tile([128, 2 * C], BF16)
        nc.sync.dma_start(fin_t[:], p["fin"][:])
        bin_t = const.tile([128, 1], F32)
        nc.sync.dma_start(bin_t[:], p["bin"][:])
        # a0 lands as 8 quarter-transfers, h2-major: after the first four, all
        # four g row-groups have their h2=0 batch halves — so the input conv
        # can run 4 matmul streams on independent PE row-tiles immediately.
        a0s = inpool.tile([128, PH * HALF * 64 * 32], BF16, tag="a0s", name="a0s")
        hcols = PH * HALF * 64 * 16        # half the cols = h2=0 part
        for g in range(4):
            nc.sync.dma_start(a0s[g * 32 : (g + 1) * 32, 0:hcols],
                              p["a0"][g * 32 : (g + 1) * 32, 0:hcols])
        # w1 rides between the a0 chunks so L1 can start right after the input
        w1t = const.tile([128, 4 * 128], BF16, name="w1t")
        nc.sync.dma_start(w1t[:], p["w1"][:])
        bias_t = {}
        bias_t[1] = const.tile([128, 4], F32, tag="bias1", name="bias1")
        nc.sync.dma_start(bias_t[1][:], p["b1"][:])
        for g in range(4):
            nc.sync.dma_start(
                a0s[g * 32 : (g + 1) * 32, hcols:],
                p["a0"][g * 32 : (g + 1) * 32, hcols:],
            )
        for lvl in range(2, NLVL):
            bias_t[lvl] = const.tile([128, LVL_NODES[lvl - 1]], F32,
                                     tag=f"bias{lvl}", name=f"bias{lvl}")
            nc.sync.dma_start(bias_t[lvl][:], p[f"b{lvl}"][:])
        b6bc_t = const.tile([128, 32 * BG], BF16, name="b6bc")
        nc.sync.dma_start(b6bc_t[:], p["b6bc"][:])
        wdt = const.tile([128, 32 * 128], BF16, name="wd")  # DMA'd during L2

        ph = 0
        a0v = a0s[:].rearrange("p (h x y) -> p h x y", h=HALF, x=64)

        # ---------------- input conv (K=32, chunk-major) ---------------------
        # X slab: [128=(y%2,c), (b, h=64, w2=32)]; b = h2*8 + g*2 + s
        X = apool.tile([128, BG * 64 * 32], BF16, tag="s0", name="x0")
        Xv = X[:].rearrange("p (b h w) -> p b h w", b=BG, h=64)
        a1 = apool.tile([128, 4 * BG * 32 * 16], BF16, tag="s1", name="a1")
        a1v = a1[:].rearrange("p (n b h w) -> p n b h w", n=4, b=BG, h=32)

        def in_quad(bls):
            # four bls on four distinct g row-groups: matmuls interleave
            # round-robin so all four PE row-tiles stream concurrently
            for xh in range(2):
                pts = {}
                for bl in bls:
                    pts[bl] = ppool.tile([128, TCOL], F32, tag="ps",
                                         padded_shape=[128, TCOL],
                                         name=f"pin{bl}_{xh}")
                for sub in range(2):
                    for bl in bls:
                        g, hi = (bl // 2) % 4, (bl // 8) * 2 + bl % 2
                        xq = xh * 2 + sub
                        rhs = a0v[g * 32 : (g + 1) * 32, hi,
                                  xq * 16 : (xq + 1) * 16, :]
                        nc.tensor.matmul(
                            pts[bl][:, sub * 512 : (sub + 1) * 512],
                            fin_t[g * 32 : (g + 1) * 32, :],
                            rhs,
                            start=True, stop=True,
                            tile_position=(g * 32, 0),
                        )
                for bl in bls:
                    # split the evict across both engines: halves the latency
                    # so the psum ring never gates the matmul stream
                    nc.scalar.activation(
                        Xv[:, bl, xh * 32 : xh * 32 + 16, :],
                        pts[bl][:, 0:512],
                        AF.Relu, bias=bin_t[:, 0:1])
                    nc.vector.tensor_scalar(
                        Xv[:, bl, xh * 32 + 16 : (xh + 1) * 32, :],
                        pts[bl][:, 512:1024],
                        bin_t[:, 0:1], 0.0, op0=ALU.add, op1=ALU.max)

        def l1_tiles(bs):
            for node in range(4):
                pt = ppool.tile([128, TCOL], F32, tag="ps",
                                padded_shape=[128, TCOL],
                                name=f"p1_{node}_{bs}")
                for x in (0, 1):
                    for q in (0, 1):
                        for sb in range(2):
                            rhs = Xv[:, bs + sb, x::2, q::2]
                            nc.tensor.matmul(
                                pt[q * 64 : (q + 1) * 64,
                                   sb * 512 : (sb + 1) * 512],
                                w1t[:, node * 128 + x * 64 :
                                    node * 128 + (x + 1) * 64],
                                rhs,
                                start=(x == 0), stop=(x == 1),
                                skip_group_check=True,
                                tile_position=(0, q * 64),
                            )
                evict(a1v[:, node, bs : bs + 2, :, :], pt[:],
                      bias_t[1][:, node : node + 1])

        # h2=0 quads first (unlocked by the first four quarter-transfers)
        for bls in ((0, 2, 4, 6), (1, 3, 5, 7), (8, 10, 12, 14), (9, 11, 13, 15)):
            in_quad(bls)
        for bs in range(0, BG, 2):
            l1_tiles(bs)

        # ---------------- levels 2..3 (q-scheme, per-node psum) --------------
        cur, cur_nodes = a1, 4
        tags = [None, "s0", "s1"]
        for lvl in (2, 3):
            n2 = LVL_NODES[lvl - 1]
            grid = int(np.sqrt(n2))
            pgrid = int(np.sqrt(cur_nodes))
            Hin = LVL_HIN[lvl - 1]
            W2in = Hin // 2
            Ho, Ko = Hin // 2, W2in // 2      # psum cols per b = Ho*Ko
            ncolb = Ho * Ko
            bper = min(BG, TCOL // ncolb)
            nsub = (bper * ncolb) // 512       # 512-col chunks per tile
            bsub = bper // nsub
            nxt = apool.tile([128, n2 * BG * ncolb], BF16,
                             tag=tags[lvl - 1], name=f"a{lvl}")
            curv = cur[:].rearrange("p (n b h w) -> p n b h w",
                                    n=cur_nodes, b=BG, h=Hin)
            nxtv = nxt[:].rearrange("p (n b h w) -> p n b h w",
                                    n=n2, b=BG, h=Ho)
            if lvl == 3:
                # dense weights: issued here so the 1 MB transfer drains
                # during L2 compute, well before anything needs the queue
                nc.sync.dma_start(wdt[:], p["wd"][:])
            # stream this level's weights in one or two chunks
            wchunk = min(n2, 16)
            for g0 in range(0, n2, wchunk):
                wlt = wpool.tile([128, 16 * 128], BF16, tag="wch",
                                 name=f"w{lvl}_{g0}")
                nc.sync.dma_start(
                    wlt[:, : wchunk * 128],
                    p[f"w{lvl}"][:, g0 * 128 : (g0 + wchunk) * 128],
                )
                # npt: nodes per psum tile (pair nodes when a node is <= 512)
                npt = 2 if bper * ncolb <= 512 else 1
                ntc = bper * ncolb                 # cols per node in the tile
                for n0 in range(g0, g0 + wchunk, npt):
                    for bs in range(0, BG, bper):
                        pt = ppool.tile([128, npt * ntc], F32, tag="ps",
                                        padded_shape=[128, TCOL],
                                        name=f"p{lvl}_{n0}_{bs}")
                        for node in range(n0, n0 + npt):
                            u, v = node // grid, node % grid
                            pn = (u // 2) * pgrid + (v // 2)
                            ln = node - g0
                            lo = (node - n0) * ntc
                            for x in (0, 1):
                                for q in (0, 1):
                                    for sb in range(nsub):
                                        b1 = bs + sb * bsub
                                        rhs = curv[:, pn, b1 : b1 + bsub,
                                                   x::2, q::2]
                                        nc.tensor.matmul(
                                            pt[q * 64 : (q + 1) * 64,
                                               lo + sb * 512 :
                                               lo + (sb + 1) * 512],
                                            wlt[:, ln * 128 + x * 64 :
                                                ln * 128 + (x + 1) * 64],
                                            rhs,
                                            start=(x == 0), stop=(x == 1),
                                            skip_group_check=True,
                                            tile_position=(0, q * 64),
                                        )
                        for node in range(n0, n0 + npt):
                            lo = (node - n0) * ntc
                            evict(
                                nxtv[:, node, bs : bs + bper, :, :],
                                pt[:, lo : lo + ntc],
                                bias_t[lvl][:, node : node + 1],
                            )
            cur, cur_nodes = nxt, n2

        # ---------------- levels 4..5 (q-scheme, node-batched psum) ----------
        for lvl in (4, 5):
            n2 = 64
            Hin = LVL_HIN[lvl - 1]
            W2in = Hin // 2
            Ho, Ko = Hin // 2, W2in // 2
            ncoln = BG * Ho * max(Ko, 1)       # cols per node (Ko>=1)
            gper = min(TCOL // ncoln, 16)      # nodes per psum tile
            nxt = apool.tile([128, n2 * ncoln], BF16,
                             tag=("s0" if lvl == 4 else "s1"), name=f"a{lvl}")
            curv = cur[:].rearrange("p (n b h w) -> p n b h w",
                                    n=64, b=BG, h=Hin)
            nxtv = nxt[:].rearrange("p (n c) -> p n c", n=n2)
            for g0 in range(0, n2, 16):
                wlt = wpool.tile([128, 16 * 128], BF16, tag="wch",
                                 name=f"w{lvl}_{g0}")
                nc.sync.dma_start(
                    wlt[:], p[f"w{lvl}"][:, g0 * 128 : (g0 + 16) * 128]
                )
                for t0 in range(g0, g0 + 16, gper):
                    pt = ppool.tile([128, gper * ncoln], F32, tag="ps",
                                    padded_shape=[128, TCOL],
                                    name=f"p{lvl}_{t0}")
                    for node in range(t0, t0 + gper):
                        ln, lt = node - g0, node - t0
                        for x in (0, 1):
                            for q in (0, 1):
                                rhs = curv[:, node, :, x::2, q::2]
                                nc.tensor.matmul(
                                    pt[q * 64 : (q + 1) * 64,
                                       lt * ncoln : (lt + 1) * ncoln],
                                    wlt[:, ln * 128 + x * 64 :
                                        ln * 128 + (x + 1) * 64],
                                    rhs,
                                    start=(x == 0), stop=(x == 1),
                                    skip_group_check=True,
                                    tile_position=(0, q * 64),
                                )
                    # batched 2-pass evict: add broadcast bias, then relu
                    tmp = tpool.tile([128, TCOL], BF16, tag="etmp",
                                     name=f"t{lvl}_{t0}")
                    bias_ap = bias_t[lvl][:, t0 : t0 + gper].unsqueeze(2) \
                        .broadcast_to([128, gper, ncoln])
                    ptv = pt[:].rearrange("p (n c) -> p n c", n=gper)
                    tv = tmp[:, : gper * ncoln].rearrange(
                        "p (n c) -> p n c", n=gper)
                    nc.vector.tensor_tensor(tv, ptv, bias_ap, op=ALU.add)
                    nc.scalar.activation(
                        nxtv[:, t0 : t0 + gper, :],
                        tv, AF.Relu,
                    )
            cur = nxt

        # ------- level 6 + dense + output, pipelined in two pair-halves ------
        # L6: node pairs -> F [128=(s,c), (pair, b)]; dense row-tiled K=64.
        F = fpool.tile([128, 32 * BG], BF16, tag="feats", name="f6")
        Fv = F[:].rearrange("p (n b) -> p n b", n=32)
        curv = cur[:].rearrange("p (n b h) -> p n b h", n=64, b=BG)
        t2s = fpool.tile([128, NK * NK * BG], F32, tag="t2s", name="t2s")
        t2sv = t2s[:].rearrange("m (n b) -> m n b", n=NK * NK)
        # hoist all w6 chunk DMAs so the second half never waits on weights
        w6ts = {}
        for g0 in range(0, 64, 16):
            w6ts[g0] = wpool.tile([128, 16 * 128], BF16, tag="wch",
                                  name=f"w6_{g0}")
            nc.sync.dma_start(
                w6ts[g0][:], p["w6"][:, g0 * 128 : (g0 + 16) * 128]
            )
        # both L6 halves' matmuls first (half 1 hides half 0's F eviction),
        # then the two dense halves (half 1 hides half 0's output copies)
        def l6_half(hf):
            pt6 = ppool.tile([128, 16 * BG], F32, tag="ps",
                             padded_shape=[128, TCOL], name=f"p6_{hf}")
            for g0 in (hf * 32, hf * 32 + 16):
                w6t = w6ts[g0]
                for node in range(g0, g0 + 16):
                    pr, s = node // 2, node % 2
                    ln = node - g0
                    lp = pr - hf * 16
                    for x in (0, 1):
                        rhs = curv[:, node, :, x]
                        nc.tensor.matmul(
                            pt6[s * 64 : (s + 1) * 64,
                                lp * BG : (lp + 1) * BG],
                            w6t[:, ln * 128 + x * 64 : ln * 128 + (x + 1) * 64],
                            rhs,
                            start=(x == 0), stop=(x == 1),
                            skip_group_check=True,
                            tile_position=(0, s * 64),
                        )
            tmp6 = tpool.tile([128, 16 * BG], BF16, tag="etmp", name=f"t6_{hf}")
            nc.vector.tensor_tensor(tmp6[:], pt6[:],
                                    b6bc_t[:, hf * 256 : (hf + 1) * 256],
                                    op=ALU.add)
            nc.scalar.activation(F[:, hf * 256 : (hf + 1) * 256], tmp6[:],
                                 AF.Relu)

        def dense_quarter(qt):
            # quartering shortens the post-matmul tail: each quarter's
            # copies + output DMA pipeline under the next quarter's matmuls
            ptd = {}
            for s in (0, 1):
                ptd[s] = ppool.tile([128, 8 * BG], F32, tag="ps",
                                    padded_shape=[128, TCOL], name=f"pd{qt}_{s}")
            for pr in range(qt * 8, qt * 8 + 8):
                lp = pr - qt * 8
                for s in (0, 1):
                    nc.tensor.matmul(
                        ptd[s][:, lp * BG : (lp + 1) * BG],
                        wdt[s * 64 : (s + 1) * 64, pr * 128 : (pr + 1) * 128],
                        Fv[s * 64 : (s + 1) * 64, pr, :],
                        start=True, stop=True,
                        tile_position=(s * 64, 0),
                    )
            for s in (0, 1):
                dst = t2sv[:, qt * 16 + s : (qt + 1) * 16 : 2, :]
                src = ptd[s][:].rearrange("m (n b) -> m n b", n=8)
                if s == 0:
                    nc.vector.tensor_copy(dst, src)
                else:
                    nc.scalar.copy(dst, src)
            nc.sync.dma_start(
                t2[:, qt * 256 : (qt + 1) * 256],
                t2s[:, qt * 256 : (qt + 1) * 256],
            )

        l6_half(0)
        l6_half(1)
        for qt in range(4):
            dense_quarter(qt)
    nc.compile()
    return nc


# ----------------------------------------------------------------------------
# entry point
# ----------------------------------------------------------------------------

def kernel(**inputs):
    inputs = {k: np.asarray(v) for k, v in inputs.items()}
    wblobs = _prep_weights(inputs)
    nc = _build_kernel()
    in_maps = []
    for c in range(NCORES):
        m = dict(wblobs)
        m["a0"] = _prep_input(inputs["in_data"][c * BC : (c + 1) * BC])
        in_maps.append(m)
    res = run_bass_kernel_spmd(nc, in_maps, list(range(NCORES)))
    outs = [_decode_output(res.results[c]["t2"]) for c in range(NCORES)]
    return np.concatenate(outs, axis=0).astype(np.float32)


if __name__ == "__main__":
    import reference as ref

    inputs = {k: np.asarray(v) for k, v in ref.setup_inputs().items()}
    expected = np.asarray(ref.reference(**inputs))
    actual = kernel(**inputs)
    err = np.abs(actual - expected).max()
    rel = err / np.abs(expected).max()
    print("absmax:", err, "rel:", rel)
